# revision 2
# baseline (speedup 1.0000x reference)
"""Trainium2 Bass kernel for nn_ConvFCLIFNet.

Pipeline: x_seq (T=64, B=512, 1, 28, 28) -> conv2x2(valid) -> FC(729) -> LIF
scan over T -> spike sequence (T, B, 729) in {0.0, 1.0}.

Strategy
--------
- conv + FC + bias + 1/tau fold into ONE matmul: y*0.5 = x_aug @ W_aug where
  x_aug = [x_pixels(784), 1.0] and W_aug[p, o] = 0.5 * (fc_w @ C)^T (C = conv
  scatter), bias row at p=784.
- Data-parallel over 8 NeuronCores: 64 samples each.
- Matmul: W chunks stationary [128 pixels, 128 features] (f32r — full PE rate,
  ~12.5 effective mantissa bits), x^T moving [128 pixels, G*64 samples].
  PSUM output [128 features, NJ=6 chunks, G*64] -> partition dim is FEATURES,
  so the LIF state q [128, 6, 64] stays on fixed partitions all 64 steps.
- LIF scan: ONE custom DVE op per timestep:
      u = (q_prev == SENT) ? 0 : q_prev;  w = z + u
      q = (w >= 1) ? SENT : 0.5 * w
  Spike decode on ScalarE: s = Relu(q + (1 - SENT)) -> exactly 1.0 iff spiked.
- Host does only layout staging (shard, pixel-major transpose, weight fold)
  plus the final gather/decode.
"""
import numpy as np

import concourse.bacc as bacc
import concourse.mybir as mybir
import concourse.tile as tile
from concourse.bass_utils import run_bass_kernel_spmd

# ---------------------------------------------------------------- constants
T, B, H, W = 64, 512, 28, 28
NPIX = H * W            # 784
NF = 729                # fc features
NCORES = 8
BS = B // NCORES        # 64 samples per core
import os
G = int(os.environ.get("LIF_G", "8"))   # timesteps per matmul group
NG = T // G
NJ = 6                  # feature chunks of 128 (768 padded)
KT = 7                  # contraction k-tiles: 6 x 128 + 17 (784 pixels + bias)
KTAIL = NPIX + 1 - 6 * 128   # 17
NS = G * BS             # moving free size = 256
SENT = float(2 ** 20)

_CACHE = {}

# ------------------------------------------------------------ custom DVE op

def _register_lif_op():
    from concourse.dve_spec import Spec, Src0, Src1, C0, C1, Zero, One, select, eq, lower
    from concourse.dve_uop import DveOpSpec
    from concourse import dve_ops

    name = "LIF_STEP_ANT"
    for op in dve_ops.OPS:
        if op.name == name:
            return op

    def _ref(in0, in1, s0, s1, imm2=None):
        u = np.where(in1 == s0, 0.0, in1).astype(np.float32)
        w = (in0 + u).astype(np.float32)
        return np.where(w >= 1.0, np.float32(s0), (w * np.float32(s1)).astype(np.float32))

    _u = select(eq(Src1, C0), Zero, Src1)
    _w = Src0 + _u
    spec = Spec(body=select(_w >= One, C0, _w * C1), reference=_ref)

    row = dve_ops._CUSTOM_DVE_ROW_BASE + len(dve_ops.OPS)
    assert row < 0x20
    dve_ops._SUB_OPCODE_FOR_NAME[name] = row
    shas = {}
    for ver in ("v3", "v4"):
        s = DveOpSpec(name=name, opcode=row, uops=lower(spec, ver=ver), rd1_en=True)
        shas[ver] = s.sha(ver)
    op = dve_ops.DveOp(name, spec, subdim=False, uops_sha=shas)
    dve_ops.OPS.append(op)
    dve_ops.CUSTOM_DVE_SPECS[name] = spec
    return op

# ------------------------------------------------------------- device build

def _build(reps: int = 1):
    lif = _register_lif_op()
    nc = bacc.Bacc(None, target_bir_lowering=False, debug=False)
    f32, f32r = mybir.dt.float32, mybir.dt.float32r
    with tile.TileContext(nc) as tc:
        with tc.tile_pool(name="dram", bufs=1, space="DRAM") as dram, \
             tc.tile_pool(name="consts", bufs=1) as consts, \
             tc.tile_pool(name="xpool", bufs=3) as xpool, \
             tc.tile_pool(name="qpool", bufs=2) as qpool, \
             tc.tile_pool(name="spool", bufs=4) as spool, \
             tc.tile_pool(name="pspool", bufs=(2 if G <= 4 else 1), space="PSUM") as pspool:
            x_in = dram.tile([NG, NPIX + 1, NS], f32r, kind="ExternalInput",
                             name="x_in", uniquify=False)
            w_in = dram.tile([KT, 128, NJ, 128], f32r, kind="ExternalInput",
                             name="w_in", uniquify=False)
            out = dram.tile([T, 128, NJ, BS], f32, kind="ExternalOutput",
                            name="out", uniquify=False)

            wsb = consts.tile([128, KT, NJ, 128], f32r)
            nc.sync.dma_start(out=wsb[:, :, :, :], in_=w_in.rearrange("k p j m -> p k j m"))
            bias_t = consts.tile([128, 1], f32)
            nc.vector.memset(bias_t[:, :], float(1.0 - SENT))

            q = qpool.tile([128, NJ, BS], f32, name="q", tag="q")
            nc.vector.memset(q[:, :, :], 0.0)

            for rep in range(reps):
                if rep > 0:
                    # timing-only extra passes reuse state; reset q
                    q = qpool.tile([128, NJ, BS], f32, name="q", tag="q")
                    nc.vector.memset(q[:, :, :], 0.0)
                for g in range(NG):
                    x_sb = xpool.tile([128, KT, NS], f32r, name="x_sb", tag="x")
                    nc.sync.dma_start(
                        out=x_sb[:, 0:6, :],
                        in_=x_in[g, 0:768, :].rearrange("(k p) n -> p k n", p=128),
                    )
                    nc.sync.dma_start(
                        out=x_sb[0:KTAIL, 6, :],
                        in_=x_in[g, 768:NPIX + 1, :],
                    )
                    ps = pspool.tile([128, NJ, NS], f32, name="ps", tag="ps")
                    for j in range(NJ):
                        for kt in range(6):
                            nc.tensor.matmul(
                                ps[:, j, :],
                                lhsT=wsb[:, kt, j, :],
                                rhs=x_sb[:, kt, :],
                                start=(kt == 0), stop=False,
                            )
                        nc.tensor.matmul(
                            ps[:, j, :],
                            lhsT=wsb[0:KTAIL, 6, j, :],
                            rhs=x_sb[0:KTAIL, 6, :],
                            start=False, stop=True,
                        )
                    for tl in range(G):
                        t = g * G + tl
                        q2 = qpool.tile([128, NJ, BS], f32, name="q", tag="q")
                        nc.vector._custom_dve(
                            lif,
                            out=q2[:, :, :],
                            in0=ps[:, :, tl * BS:(tl + 1) * BS],
                            in1=q[:, :, :],
                            s0=SENT, s1=0.5,
                        )
                        s_sb = spool.tile([128, NJ, BS], f32, name="s_sb", tag="s")
                        nc.scalar.activation(
                            s_sb[:, :, :], q2[:, :, :],
                            mybir.ActivationFunctionType.Relu,
                            bias=bias_t[:, :], scale=1.0,
                        )
                        nc.sync.dma_start(out=out[t], in_=s_sb[:, :, :])
                        q = q2
    nc.compile()
    return nc

# --------------------------------------------------------------- host side

def _prep_weights(conv_w, fc_w, fc_b):
    """W_aug [KT,128,NJ,128]: rows = pixels (784) + bias row (784) + pad,
    cols = 768 features (729 + pad); scaled by 0.5 (tau fold)."""
    cw = conv_w.reshape(2, 2).astype(np.float32)
    fcw = fc_w.astype(np.float32).reshape(NF, 27, 27)
    tmp = np.zeros((NF, H, W), np.float32)
    for dr in range(2):
        for dc in range(2):
            tmp[:, dr:dr + 27, dc:dc + 27] += cw[dr, dc] * fcw
    w_eff = tmp.reshape(NF, NPIX)                     # [729, 784]
    w_aug = np.zeros((KT * 128, NJ * 128), np.float32)
    w_aug[:NPIX, :NF] = 0.5 * w_eff.T
    w_aug[NPIX, :NF] = 0.5 * fc_b.astype(np.float32)
    return np.ascontiguousarray(
        w_aug.reshape(KT, 128, NJ, 128))

def _prep_x(x_seq):
    """Per-core pixel-major inputs [NCORES][NG, 785, G*64]."""
    xs = np.ascontiguousarray(x_seq.reshape(T, NCORES, BS, NPIX))
    # -> [core, group, pixel, (tl, sample)]
    xt = xs.transpose(1, 0, 3, 2).reshape(NCORES, NG, G, NPIX, BS)
    xt = xt.transpose(0, 1, 3, 2, 4).reshape(NCORES, NG, NPIX, NS)
    xp = np.empty((NCORES, NG, NPIX + 1, NS), np.float32)
    xp[:, :, :NPIX, :] = xt
    xp[:, :, NPIX, :] = 1.0
    return xp

def kernel(x_seq, conv_w, fc_w, fc_b):
    if "nc" not in _CACHE:
        _CACHE["nc"] = _build(reps=1)
    nc = _CACHE["nc"]
    w_aug = _prep_weights(conv_w, fc_w, fc_b)
    xp = _prep_x(np.asarray(x_seq, dtype=np.float32))
    in_maps = [{"x_in": np.ascontiguousarray(xp[c]), "w_in": w_aug}
               for c in range(NCORES)]
    res = run_bass_kernel_spmd(nc, in_maps, core_ids=list(range(NCORES)))
    _CACHE["last_result"] = res
    full = np.empty((T, B, NF), np.float32)
    for c in range(NCORES):
        o = res.results[c]["out"]                     # [T, 128, NJ, BS]
        # feature f = j*128 + p ; sample s
        full[:, c * BS:(c + 1) * BS, :] = (
            o.transpose(0, 3, 2, 1).reshape(T, BS, NJ * 128)[:, :, :NF])
    return full



# revision 5
# speedup vs baseline: 2.2387x; 2.2387x over previous
"""Trainium2 Bass kernel for nn_ConvFCLIFNet.

Pipeline: x_seq (T=64, B=512, 1, 28, 28) -> conv2x2(valid) -> FC(729) -> LIF
scan over T -> spike sequence (T, B, 729) in {0.0, 1.0}.

Strategy
--------
- conv + FC + bias + 1/tau fold into ONE matmul: y*0.5 = x_aug @ W_aug where
  x_aug = [x_pixels(784), 1.0] and W_aug[p, o] = 0.5 * (fc_w @ C)^T (C = conv
  scatter), bias row at p=784. Rows 785..895 of W_aug are zero, so the kt=6
  k-chunk can run with all 128 partitions (garbage x rows * 0 weight = 0).
- Data-parallel over 8 NeuronCores: 64 samples each.
- Matmul: W chunks stationary [128 pixels, 128 features] (f32r), x^T moving
  [128 pixels, G*64 samples] with G=8 (moving 512 >= 256 keeps f32r at the
  full 1 cycle/row PE rate).
- PSUM is j-split into two halves ps_a (features 0..383) / ps_b (384..767),
  6KB each, so the LIF drain of one half overlaps matmuls into the other:
  the tensor engine never idles and stays at the 2.4GHz hot p-state.
- LIF scan: custom DVE op per (timestep, half):
      u = (q_prev == SENT) ? 0 : q_prev;  w = z + u
      q = (w >= 1) ? SENT : 0.5 * w
  Spike decode on ScalarE: s = Relu(q + (1 - SENT)) -> exactly 1.0 iff spiked,
  emitted as uint8 into a per-group staging tile (one 3KB/partition store per
  group instead of 64 f32 stores).
- Host does only layout staging (shard, pixel-major transpose, weight fold)
  plus the final gather/decode.
"""
import numpy as np

import concourse.bacc as bacc
import concourse.mybir as mybir
import concourse.tile as tile
from concourse.bass_utils import run_bass_kernel_spmd

# ---------------------------------------------------------------- constants
T, B, H, W = 64, 512, 28, 28
NPIX = H * W            # 784
NF = 729                # fc features
NCORES = 8
BS = B // NCORES        # 64 samples per core
G = 8                   # timesteps per matmul group
NG = T // G             # 8 groups
NJ = 6                  # feature chunks of 128 (768 padded)
NJH = 3                 # feature chunks per PSUM half
KT = 7                  # contraction k-tiles of 128 (785 real rows + zero pad)
NS = G * BS             # moving free size = 512
SENT = float(2 ** 20)

_CACHE = {}

# ------------------------------------------------------------ custom DVE op

def _register_lif_op():
    from concourse.dve_spec import Spec, Src0, Src1, C0, C1, Zero, One, select, eq, lower
    from concourse.dve_uop import DveOpSpec
    from concourse import dve_ops

    name = "LIF_STEP_ANT"
    for op in dve_ops.OPS:
        if op.name == name:
            return op

    def _ref(in0, in1, s0, s1, imm2=None):
        u = np.where(in1 == s0, 0.0, in1).astype(np.float32)
        w = (in0 + u).astype(np.float32)
        return np.where(w >= 1.0, np.float32(s0), (w * np.float32(s1)).astype(np.float32))

    _u = select(eq(Src1, C0), Zero, Src1)
    _w = Src0 + _u
    spec = Spec(body=select(_w >= One, C0, _w * C1), reference=_ref)

    row = dve_ops._CUSTOM_DVE_ROW_BASE + len(dve_ops.OPS)
    assert row < 0x20
    dve_ops._SUB_OPCODE_FOR_NAME[name] = row
    shas = {}
    for ver in ("v3", "v4"):
        s = DveOpSpec(name=name, opcode=row, uops=lower(spec, ver=ver), rd1_en=True)
        shas[ver] = s.sha(ver)
    op = dve_ops.DveOp(name, spec, subdim=False, uops_sha=shas)
    dve_ops.OPS.append(op)
    dve_ops.CUSTOM_DVE_SPECS[name] = spec
    return op

# ------------------------------------------------------------- device build

def _build():
    lif = _register_lif_op()
    nc = bacc.Bacc(None, target_bir_lowering=False, debug=False)
    f32, f32r, u8 = mybir.dt.float32, mybir.dt.float32r, mybir.dt.uint8
    with tile.TileContext(nc) as tc:
        with tc.tile_pool(name="dram", bufs=1, space="DRAM") as dram, \
             tc.tile_pool(name="consts", bufs=1) as consts, \
             tc.tile_pool(name="xpool", bufs=3) as xpool, \
             tc.tile_pool(name="qa", bufs=2) as qa_pool, \
             tc.tile_pool(name="qb", bufs=2) as qb_pool, \
             tc.tile_pool(name="spool", bufs=2) as spool, \
             tc.tile_pool(name="psa", bufs=1, space="PSUM") as psa_pool, \
             tc.tile_pool(name="psb", bufs=1, space="PSUM") as psb_pool:
            # x_in: partition-major, [group][partition p][k-chunk kt][sample n]
            # row kt*128+p of x_aug; rows 768..784 only exist for kt=6, p<17.
            x_in = dram.tile([NG, 128, KT, NS], f32r, kind="ExternalInput",
                             name="x_in", uniquify=False)
            # w_in: pre-permuted on host to [p][kt][j][m]
            w_in = dram.tile([128, KT, NJ, 128], f32r, kind="ExternalInput",
                             name="w_in", uniquify=False)
            # out: partition-major u8 spikes, [group][p][tl][j][sample]
            out = dram.tile([NG, 128, G, NJ, BS], u8, kind="ExternalOutput",
                            name="out", uniquify=False)

            wsb = consts.tile([128, KT, NJ, 128], f32r)
            nc.sync.dma_start(out=wsb[:, :, :, :], in_=w_in[:, :, :, :])
            bias_t = consts.tile([128, 1], f32)
            nc.vector.memset(bias_t[:, :], float(1.0 - SENT))

            q_a = qa_pool.tile([128, NJH, BS], f32, name="q_a", tag="qa")
            nc.vector.memset(q_a[:, :, :], 0.0)
            q_b = qb_pool.tile([128, NJH, BS], f32, name="q_b", tag="qb")
            nc.vector.memset(q_b[:, :, :], 0.0)

            for g in range(NG):
                x_sb = xpool.tile([128, KT, NS], f32r, name="x_sb", tag="x")
                # one DMA per group: 14KB contiguous per partition (rows
                # 785..895 are zero-padded host-side)
                nc.sync.dma_start(out=x_sb[:, :, :], in_=x_in[g])

                ps_a = psa_pool.tile([128, NJH, NS], f32, name="ps_a", tag="psa")
                ps_b = psb_pool.tile([128, NJH, NS], f32, name="ps_b", tag="psb")
                for j in range(NJ):
                    ps = ps_a if j < NJH else ps_b
                    jj = j if j < NJH else j - NJH
                    for kt in range(KT):
                        nc.tensor.matmul(
                            ps[:, jj, :],
                            lhsT=wsb[:, kt, j, :],
                            rhs=x_sb[:, kt, :],
                            start=(kt == 0), stop=(kt == KT - 1),
                        )

                s_g = spool.tile([128, G, NJ, BS], u8, name="s_g", tag="s")
                for tl in range(G):
                    q2_a = qa_pool.tile([128, NJH, BS], f32, name="q_a", tag="qa")
                    nc.vector._custom_dve(
                        lif,
                        out=q2_a[:, :, :],
                        in0=ps_a[:, :, tl * BS:(tl + 1) * BS],
                        in1=q_a[:, :, :],
                        s0=SENT, s1=0.5,
                    )
                    nc.scalar.activation(
                        s_g[:, tl, 0:NJH, :], q2_a[:, :, :],
                        mybir.ActivationFunctionType.Relu,
                        bias=bias_t[:, :], scale=1.0,
                    )
                    q_a = q2_a

                    q2_b = qb_pool.tile([128, NJH, BS], f32, name="q_b", tag="qb")
                    nc.vector._custom_dve(
                        lif,
                        out=q2_b[:, :, :],
                        in0=ps_b[:, :, tl * BS:(tl + 1) * BS],
                        in1=q_b[:, :, :],
                        s0=SENT, s1=0.5,
                    )
                    nc.scalar.activation(
                        s_g[:, tl, NJH:NJ, :], q2_b[:, :, :],
                        mybir.ActivationFunctionType.Relu,
                        bias=bias_t[:, :], scale=1.0,
                    )
                    q_b = q2_b

                nc.sync.dma_start(out=out[g], in_=s_g[:, :, :, :])
    nc.compile()
    return nc

# --------------------------------------------------------------- host side

def _prep_weights(conv_w, fc_w, fc_b):
    """W_aug permuted to [128 p, KT, NJ, 128 m]: row kt*128+p = pixel (or bias
    at 784, zero pad above), col j*128+m = feature; scaled by 0.5 (tau fold)."""
    cw = conv_w.reshape(2, 2).astype(np.float32)
    fcw = fc_w.astype(np.float32).reshape(NF, 27, 27)
    tmp = np.zeros((NF, H, W), np.float32)
    for dr in range(2):
        for dc in range(2):
            tmp[:, dr:dr + 27, dc:dc + 27] += cw[dr, dc] * fcw
    w_eff = tmp.reshape(NF, NPIX)                     # [729, 784]
    w_aug = np.zeros((KT * 128, NJ * 128), np.float32)
    w_aug[:NPIX, :NF] = 0.5 * w_eff.T
    w_aug[NPIX, :NF] = 0.5 * fc_b.astype(np.float32)
    # [KT*128, NJ*128] -> [KT, 128, NJ, 128] -> [128, KT, NJ, 128]
    return np.ascontiguousarray(
        w_aug.reshape(KT, 128, NJ, 128).transpose(1, 0, 2, 3))

def _prep_x(x_seq):
    """Per-core pixel-major inputs [NCORES][NG, 128, KT, NS]."""
    xs = np.ascontiguousarray(x_seq.reshape(T, NCORES, BS, NPIX))
    # -> [core, group, pixel, (tl, sample)]
    xt = xs.transpose(1, 0, 3, 2).reshape(NCORES, NG, G, NPIX, BS)
    xt = xt.transpose(0, 1, 3, 2, 4).reshape(NCORES, NG, NPIX, NS)
    xp = np.zeros((NCORES, NG, KT * 128, NS), np.float32)
    xp[:, :, :NPIX, :] = xt
    xp[:, :, NPIX, :] = 1.0
    # rows -> [128, KT]: row kt*128+p at [p, kt]
    xp = xp.reshape(NCORES, NG, KT, 128, NS).transpose(0, 1, 3, 2, 4)
    return np.ascontiguousarray(xp)

def kernel(x_seq, conv_w, fc_w, fc_b):
    if "nc" not in _CACHE:
        _CACHE["nc"] = _build()
    nc = _CACHE["nc"]
    w_aug = _prep_weights(conv_w, fc_w, fc_b)
    xp = _prep_x(np.asarray(x_seq, dtype=np.float32))
    in_maps = [{"x_in": np.ascontiguousarray(xp[c]), "w_in": w_aug}
               for c in range(NCORES)]
    res = run_bass_kernel_spmd(nc, in_maps, core_ids=list(range(NCORES)))
    _CACHE["last_result"] = res
    full = np.empty((T, B, NF), np.float32)
    for c in range(NCORES):
        o = res.results[c]["out"]                     # [NG, 128, G, NJ, BS] u8
        # (g, p, tl, j, s) -> (g, tl, s, j, p); feature f = j*128 + p
        full[:, c * BS:(c + 1) * BS, :] = (
            o.transpose(0, 2, 4, 3, 1).reshape(T, BS, NJ * 128)[:, :, :NF]
            .astype(np.float32))
    return full


# revision 7
# speedup vs baseline: 2.2780x; 1.0175x over previous
"""Trainium2 Bass kernel for nn_ConvFCLIFNet.

Pipeline: x_seq (T=64, B=512, 1, 28, 28) -> conv2x2(valid) -> FC(729) -> LIF
scan over T -> spike sequence (T, B, 729) in {0.0, 1.0}.

Strategy
--------
- conv + FC + bias + 1/tau fold into ONE matmul: y*0.5 = x_aug @ W_aug where
  x_aug = [x_pixels(784), 1.0] and W_aug[p, o] = 0.5 * (fc_w @ C)^T (C = conv
  scatter), bias row at p=784. Rows 785..895 of W_aug are zero, so the kt=6
  k-chunk can run with all 128 partitions (garbage x rows * 0 weight = 0).
- Data-parallel over 8 NeuronCores: 64 samples each.
- Matmul: W chunks stationary [128 pixels, 128 features] (f32r), x^T moving
  [128 pixels, G*64 samples] with G=8 (moving 512 >= 256 keeps f32r at the
  full 1 cycle/row PE rate).
- PSUM is j-split into two halves ps_a (features 0..383) / ps_b (384..767),
  6KB each, so the LIF drain of one half overlaps matmuls into the other:
  the tensor engine never idles and stays at the 2.4GHz hot p-state.
- LIF scan: custom DVE op per (timestep, half):
      u = (q_prev == SENT) ? 0 : q_prev;  w = z + u
      q = (w >= 1) ? SENT : 0.5 * w
  Spike decode on ScalarE: s = Relu(q + (1 - SENT)) -> exactly 1.0 iff spiked,
  emitted as uint8 into a per-group staging tile (one 3KB/partition store per
  group instead of 64 f32 stores).
- Host does only layout staging (shard, pixel-major transpose, weight fold)
  plus the final gather/decode.
"""
import numpy as np

import concourse.bacc as bacc
import concourse.mybir as mybir
import concourse.tile as tile
from concourse.bass_utils import run_bass_kernel_spmd

# ---------------------------------------------------------------- constants
T, B, H, W = 64, 512, 28, 28
NPIX = H * W            # 784
NF = 729                # fc features
NCORES = 8
BS = B // NCORES        # 64 samples per core
G = 8                   # timesteps per matmul group
NG = T // G             # 8 groups
NJ = 6                  # feature chunks of 128 (768 padded)
NJH = 3                 # feature chunks per PSUM half
KT = 7                  # contraction k-tiles of 128 (785 real rows + zero pad)
NS = G * BS             # moving free size = 512
SENT = float(2 ** 20)

_CACHE = {}

# ------------------------------------------------------------ custom DVE op

def _register_lif_op():
    from concourse.dve_spec import Spec, Src0, Src1, C0, C1, Zero, One, select, eq, lower
    from concourse.dve_uop import DveOpSpec
    from concourse import dve_ops

    name = "LIF_STEP_ANT"
    for op in dve_ops.OPS:
        if op.name == name:
            return op

    def _ref(in0, in1, s0, s1, imm2=None):
        u = np.where(in1 == s0, 0.0, in1).astype(np.float32)
        w = (in0 + u).astype(np.float32)
        return np.where(w >= 1.0, np.float32(s0), (w * np.float32(s1)).astype(np.float32))

    _u = select(eq(Src1, C0), Zero, Src1)
    _w = Src0 + _u
    spec = Spec(body=select(_w >= One, C0, _w * C1), reference=_ref)

    row = dve_ops._CUSTOM_DVE_ROW_BASE + len(dve_ops.OPS)
    assert row < 0x20
    dve_ops._SUB_OPCODE_FOR_NAME[name] = row
    shas = {}
    for ver in ("v3", "v4"):
        s = DveOpSpec(name=name, opcode=row, uops=lower(spec, ver=ver), rd1_en=True)
        shas[ver] = s.sha(ver)
    op = dve_ops.DveOp(name, spec, subdim=False, uops_sha=shas)
    dve_ops.OPS.append(op)
    dve_ops.CUSTOM_DVE_SPECS[name] = spec
    return op

# ------------------------------------------------------------- device build

def _build():
    lif = _register_lif_op()
    nc = bacc.Bacc(None, target_bir_lowering=False, debug=False)
    f32, f32r, u8 = mybir.dt.float32, mybir.dt.float32r, mybir.dt.uint8
    with tile.TileContext(nc) as tc:
        with tc.tile_pool(name="dram", bufs=1, space="DRAM") as dram, \
             tc.tile_pool(name="consts", bufs=1) as consts, \
             tc.tile_pool(name="xpool", bufs=3) as xpool, \
             tc.tile_pool(name="qa", bufs=2) as qa_pool, \
             tc.tile_pool(name="qb", bufs=2) as qb_pool, \
             tc.tile_pool(name="spool", bufs=2) as spool, \
             tc.tile_pool(name="psa", bufs=1, space="PSUM") as psa_pool, \
             tc.tile_pool(name="psb", bufs=1, space="PSUM") as psb_pool:
            # x_in: partition-major, [group][partition p][k-chunk kt][sample n]
            # row kt*128+p of x_aug; rows 768..784 only exist for kt=6, p<17.
            x_in = dram.tile([NG, 128, KT, NS], f32r, kind="ExternalInput",
                             name="x_in", uniquify=False)
            # w_in: pre-permuted on host to [p][kt][j][m]
            w_in = dram.tile([128, KT, NJ, 128], f32r, kind="ExternalInput",
                             name="w_in", uniquify=False)
            # out: partition-major u8 spikes, [group][p][tl][j][sample]
            out = dram.tile([NG, 128, G, NJ, BS], u8, kind="ExternalOutput",
                            name="out", uniquify=False)

            wsb = consts.tile([128, KT, NJ, 128], f32r)
            bias_t = consts.tile([128, 1], f32)
            nc.vector.memset(bias_t[:, :], float(1.0 - SENT))

            # g=0 prologue: interleave x/w transfers at kt granularity so the
            # first matmul column can start after ~2 chunks instead of after
            # the full 4.6MB; later groups prefetch under compute anyway.
            x_sb0 = xpool.tile([128, KT, NS], f32r, name="x_sb", tag="x")
            for kt in range(KT):
                nc.sync.dma_start(out=x_sb0[:, kt, :], in_=x_in[0, :, kt, :])
                nc.sync.dma_start(out=wsb[:, kt, :, :], in_=w_in[:, kt, :, :])

            # per-group q history [128, tl, jchunk, sample]; slice G-1 of the
            # previous group's tile seeds the LIF chain (memset -> v0 = 0).
            q_a = qa_pool.tile([128, G, NJH, BS], f32, name="q_a", tag="qa")
            nc.vector.memset(q_a[:, :, :, :], 0.0)
            q_b = qb_pool.tile([128, G, NJH, BS], f32, name="q_b", tag="qb")
            nc.vector.memset(q_b[:, :, :, :], 0.0)

            for g in range(NG):
                if g == 0:
                    x_sb = x_sb0
                else:
                    x_sb = xpool.tile([128, KT, NS], f32r, name="x_sb", tag="x")
                    # one DMA per group: 14KB contiguous per partition (rows
                    # 785..895 are zero-padded host-side)
                    nc.sync.dma_start(out=x_sb[:, :, :], in_=x_in[g])

                ps_a = psa_pool.tile([128, NJH, NS], f32, name="ps_a", tag="psa")
                ps_b = psb_pool.tile([128, NJH, NS], f32, name="ps_b", tag="psb")
                for j in range(NJ):
                    ps = ps_a if j < NJH else ps_b
                    jj = j if j < NJH else j - NJH
                    for kt in range(KT):
                        nc.tensor.matmul(
                            ps[:, jj, :],
                            lhsT=wsb[:, kt, j, :],
                            rhs=x_sb[:, kt, :],
                            start=(kt == 0), stop=(kt == KT - 1),
                        )

                s_g = spool.tile([128, G, NJ, BS], u8, name="s_g", tag="s")
                q2_a = qa_pool.tile([128, G, NJH, BS], f32, name="q_a", tag="qa")
                q2_b = qb_pool.tile([128, G, NJH, BS], f32, name="q_b", tag="qb")
                for tl in range(G):
                    nc.vector._custom_dve(
                        lif,
                        out=q2_a[:, tl, :, :],
                        in0=ps_a[:, :, tl * BS:(tl + 1) * BS],
                        in1=(q_a[:, G - 1, :, :] if tl == 0
                             else q2_a[:, tl - 1, :, :]),
                        s0=SENT, s1=0.5,
                    )
                    nc.vector._custom_dve(
                        lif,
                        out=q2_b[:, tl, :, :],
                        in0=ps_b[:, :, tl * BS:(tl + 1) * BS],
                        in1=(q_b[:, G - 1, :, :] if tl == 0
                             else q2_b[:, tl - 1, :, :]),
                        s0=SENT, s1=0.5,
                    )
                # one spike decode per group half over the whole q history
                nc.scalar.activation(
                    s_g[:, :, 0:NJH, :], q2_a[:, :, :, :],
                    mybir.ActivationFunctionType.Relu,
                    bias=bias_t[:, :], scale=1.0,
                )
                nc.scalar.activation(
                    s_g[:, :, NJH:NJ, :], q2_b[:, :, :, :],
                    mybir.ActivationFunctionType.Relu,
                    bias=bias_t[:, :], scale=1.0,
                )
                q_a, q_b = q2_a, q2_b

                nc.sync.dma_start(out=out[g], in_=s_g[:, :, :, :])
    nc.compile()
    return nc

# --------------------------------------------------------------- host side

def _prep_weights(conv_w, fc_w, fc_b):
    """W_aug permuted to [128 p, KT, NJ, 128 m]: row kt*128+p = pixel (or bias
    at 784, zero pad above), col j*128+m = feature; scaled by 0.5 (tau fold)."""
    cw = conv_w.reshape(2, 2).astype(np.float32)
    fcw = fc_w.astype(np.float32).reshape(NF, 27, 27)
    tmp = np.zeros((NF, H, W), np.float32)
    for dr in range(2):
        for dc in range(2):
            tmp[:, dr:dr + 27, dc:dc + 27] += cw[dr, dc] * fcw
    w_eff = tmp.reshape(NF, NPIX)                     # [729, 784]
    w_aug = np.zeros((KT * 128, NJ * 128), np.float32)
    w_aug[:NPIX, :NF] = 0.5 * w_eff.T
    w_aug[NPIX, :NF] = 0.5 * fc_b.astype(np.float32)
    # [KT*128, NJ*128] -> [KT, 128, NJ, 128] -> [128, KT, NJ, 128]
    return np.ascontiguousarray(
        w_aug.reshape(KT, 128, NJ, 128).transpose(1, 0, 2, 3))

def _prep_x(x_seq):
    """Per-core pixel-major inputs [NCORES][NG, 128, KT, NS]."""
    xs = np.ascontiguousarray(x_seq.reshape(T, NCORES, BS, NPIX))
    # -> [core, group, pixel, (tl, sample)]
    xt = xs.transpose(1, 0, 3, 2).reshape(NCORES, NG, G, NPIX, BS)
    xt = xt.transpose(0, 1, 3, 2, 4).reshape(NCORES, NG, NPIX, NS)
    xp = np.zeros((NCORES, NG, KT * 128, NS), np.float32)
    xp[:, :, :NPIX, :] = xt
    xp[:, :, NPIX, :] = 1.0
    # rows -> [128, KT]: row kt*128+p at [p, kt]
    xp = xp.reshape(NCORES, NG, KT, 128, NS).transpose(0, 1, 3, 2, 4)
    return np.ascontiguousarray(xp)

def kernel(x_seq, conv_w, fc_w, fc_b):
    if "nc" not in _CACHE:
        _CACHE["nc"] = _build()
    nc = _CACHE["nc"]
    w_aug = _prep_weights(conv_w, fc_w, fc_b)
    xp = _prep_x(np.asarray(x_seq, dtype=np.float32))
    in_maps = [{"x_in": np.ascontiguousarray(xp[c]), "w_in": w_aug}
               for c in range(NCORES)]
    res = run_bass_kernel_spmd(nc, in_maps, core_ids=list(range(NCORES)))
    _CACHE["last_result"] = res
    full = np.empty((T, B, NF), np.float32)
    for c in range(NCORES):
        o = res.results[c]["out"]                     # [NG, 128, G, NJ, BS] u8
        # (g, p, tl, j, s) -> (g, tl, s, j, p); feature f = j*128 + p
        full[:, c * BS:(c + 1) * BS, :] = (
            o.transpose(0, 2, 4, 3, 1).reshape(T, BS, NJ * 128)[:, :, :NF]
            .astype(np.float32))
    return full


# revision 13
# speedup vs baseline: 2.3450x; 1.0294x over previous
"""Trainium2 Bass kernel for nn_ConvFCLIFNet.

Pipeline: x_seq (T=64, B=512, 1, 28, 28) -> conv2x2(valid) -> FC(729) -> LIF
scan over T -> spike sequence (T, B, 729) in {0.0, 1.0}.

Strategy
--------
- conv + FC + bias + 1/tau fold into ONE matmul: y*0.5 = x_aug @ W_aug where
  x_aug = [x_pixels(784), 1.0] and W_aug[p, o] = 0.5 * (fc_w @ C)^T (C = conv
  scatter), bias row at p=784. Rows 785..895 of W_aug are zero, so the kt=6
  k-chunk can run with all 128 partitions (garbage x rows * 0 weight = 0).
- Data-parallel over 8 NeuronCores: 64 samples each.
- Matmul: W chunks stationary [128 pixels, 128 features] (f32r), x^T moving
  [128 pixels, G*64 samples] with G=8 (moving 512 >= 256 keeps f32r at the
  full 1 cycle/row PE rate).
- PSUM is j-split into two halves ps_a (features 0..383) / ps_b (384..767),
  6KB each, so the LIF drain of one half overlaps matmuls into the other:
  the tensor engine never idles and stays at the 2.4GHz hot p-state.
- LIF scan: custom DVE op per (timestep, half):
      u = (q_prev == SENT) ? 0 : q_prev;  w = z + u
      q = (w >= 1) ? SENT : 0.5 * w
  Spike decode on ScalarE: s = Relu(q + (1 - SENT)) -> exactly 1.0 iff spiked,
  emitted as uint8 into a per-group staging tile (one 3KB/partition store per
  group instead of 64 f32 stores).
- Host does only layout staging (shard, pixel-major transpose, weight fold)
  plus the final gather/decode.
"""
import numpy as np

import concourse.bacc as bacc
import concourse.mybir as mybir
import concourse.tile as tile
from concourse.bass_utils import run_bass_kernel_spmd

# ---------------------------------------------------------------- constants
T, B, H, W = 64, 512, 28, 28
NPIX = H * W            # 784
NF = 729                # fc features
NCORES = 8
BS = B // NCORES        # 64 samples per core
G = 8                   # timesteps per matmul group
NG = T // G             # 8 groups
NJ = 6                  # feature chunks of 128 (768 padded)
NJH = 3                 # feature chunks per PSUM half
KT = 7                  # contraction k-tiles of 128 (785 real rows + zero pad)
NS = G * BS             # moving free size = 512
SENT = float(2 ** 20)

_CACHE = {}

# ------------------------------------------------------------ custom DVE op

def _register_lif_op():
    from concourse.dve_spec import Spec, Src0, Src1, C0, C1, Zero, One, select, eq, lower
    from concourse.dve_uop import DveOpSpec
    from concourse import dve_ops

    name = "LIF_STEP_ANT"
    for op in dve_ops.OPS:
        if op.name == name:
            return op

    def _ref(in0, in1, s0, s1, imm2=None):
        u = np.where(in1 == s0, 0.0, in1).astype(np.float32)
        w = (in0 + u).astype(np.float32)
        return np.where(w >= 1.0, np.float32(s0), (w * np.float32(s1)).astype(np.float32))

    _u = select(eq(Src1, C0), Zero, Src1)
    _w = Src0 + _u
    spec = Spec(body=select(_w >= One, C0, _w * C1), reference=_ref)

    row = dve_ops._CUSTOM_DVE_ROW_BASE + len(dve_ops.OPS)
    assert row < 0x20
    dve_ops._SUB_OPCODE_FOR_NAME[name] = row
    shas = {}
    for ver in ("v3", "v4"):
        s = DveOpSpec(name=name, opcode=row, uops=lower(spec, ver=ver), rd1_en=True)
        shas[ver] = s.sha(ver)
    op = dve_ops.DveOp(name, spec, subdim=False, uops_sha=shas)
    dve_ops.OPS.append(op)
    dve_ops.CUSTOM_DVE_SPECS[name] = spec
    return op

# ------------------------------------------------------------- device build

def _build():
    lif = _register_lif_op()
    nc = bacc.Bacc(None, target_bir_lowering=False, debug=False)
    f32, f32r, u8 = mybir.dt.float32, mybir.dt.float32r, mybir.dt.uint8
    with tile.TileContext(nc) as tc:
        with tc.tile_pool(name="dram", bufs=1, space="DRAM") as dram, \
             tc.tile_pool(name="consts", bufs=1) as consts, \
             tc.tile_pool(name="xpool", bufs=3) as xpool, \
             tc.tile_pool(name="qa", bufs=2) as qa_pool, \
             tc.tile_pool(name="qb", bufs=2) as qb_pool, \
             tc.tile_pool(name="spool", bufs=2) as spool, \
             tc.tile_pool(name="psa", bufs=1, space="PSUM") as psa_pool, \
             tc.tile_pool(name="psb", bufs=1, space="PSUM") as psb_pool:
            # x_in: partition-major, [group][partition p][k-chunk kt][sample n]
            # row kt*128+p of x_aug; rows 768..784 only exist for kt=6, p<17.
            x_in = dram.tile([NG, 128, KT, NS], f32r, kind="ExternalInput",
                             name="x_in", uniquify=False)
            # w_in: pre-permuted on host to [p][j][kt][m] (j-major so the
            # first output column's weights arrive in one small DMA)
            w_in = dram.tile([128, NJ, KT, 128], f32r, kind="ExternalInput",
                             name="w_in", uniquify=False)
            # out: partition-major u8 spikes, [group][p][half][tl][jh][sample]
            out = dram.tile([NG, 128, 2, G, NJH, BS], u8, kind="ExternalOutput",
                            name="out", uniquify=False)

            wsb = consts.tile([128, NJ, KT, 128], f32r)
            bias_t = consts.tile([128, 1], f32)
            nc.vector.memset(bias_t[:, :], float(1.0 - SENT))

            # g=0 prologue: x chunks and j-major weight slices interleaved so
            # the first matmul column starts after ~2 small transfers and the
            # j columns chase the weight DMAs; later groups prefetch under
            # compute anyway.
            x_sb0 = xpool.tile([128, KT, NS], f32r, name="x_sb", tag="x")
            nc.sync.dma_start(out=x_sb0[:, 0, :], in_=x_in[0, :, 0, :])
            nc.sync.dma_start(out=wsb[:, 0, :, :], in_=w_in[:, 0, :, :])
            for kt in range(1, KT):
                nc.sync.dma_start(out=x_sb0[:, kt, :], in_=x_in[0, :, kt, :])
            for j in range(1, NJ):
                nc.sync.dma_start(out=wsb[:, j, :, :], in_=w_in[:, j, :, :])

            # per-group q history [128, tl, jchunk, sample]; slice G-1 of the
            # previous group's tile seeds the LIF chain (memset -> v0 = 0).
            q_a = qa_pool.tile([128, G, NJH, BS], f32, name="q_a", tag="qa")
            nc.vector.memset(q_a[:, :, :, :], 0.0)
            q_b = qb_pool.tile([128, G, NJH, BS], f32, name="q_b", tag="qb")
            nc.vector.memset(q_b[:, :, :, :], 0.0)

            for g in range(NG):
                if g == 0:
                    x_sb = x_sb0
                else:
                    x_sb = xpool.tile([128, KT, NS], f32r, name="x_sb", tag="x")
                    # one DMA per group: 14KB contiguous per partition (rows
                    # 785..895 are zero-padded host-side)
                    nc.sync.dma_start(out=x_sb[:, :, :], in_=x_in[g])

                ps_a = psa_pool.tile([128, NJH, NS], f32, name="ps_a", tag="psa")
                ps_b = psb_pool.tile([128, NJH, NS], f32, name="ps_b", tag="psb")
                for j in range(NJ):
                    ps = ps_a if j < NJH else ps_b
                    jj = j if j < NJH else j - NJH
                    for kt in range(KT):
                        nc.tensor.matmul(
                            ps[:, jj, :],
                            lhsT=wsb[:, j, kt, :],
                            rhs=x_sb[:, kt, :],
                            start=(kt == 0), stop=(kt == KT - 1),
                        )

                s_a = spool.tile([128, G, NJH, BS], u8, name="s_a", tag="sa")
                s_b = spool.tile([128, G, NJH, BS], u8, name="s_b", tag="sb")
                q2_a = qa_pool.tile([128, G, NJH, BS], f32, name="q_a", tag="qa")
                q2_b = qb_pool.tile([128, G, NJH, BS], f32, name="q_b", tag="qb")
                for tl in range(G):
                    nc.vector._custom_dve(
                        lif,
                        out=q2_a[:, tl, :, :],
                        in0=ps_a[:, :, tl * BS:(tl + 1) * BS],
                        in1=(q_a[:, G - 1, :, :] if tl == 0
                             else q2_a[:, tl - 1, :, :]),
                        s0=SENT, s1=0.5,
                    )
                    nc.vector._custom_dve(
                        lif,
                        out=q2_b[:, tl, :, :],
                        in0=ps_b[:, :, tl * BS:(tl + 1) * BS],
                        in1=(q_b[:, G - 1, :, :] if tl == 0
                             else q2_b[:, tl - 1, :, :]),
                        s0=SENT, s1=0.5,
                    )
                # one spike decode + store per group half; the a-half ships
                # while the b-chain is still draining
                nc.scalar.activation(
                    s_a[:, :, :, :], q2_a[:, :, :, :],
                    mybir.ActivationFunctionType.Relu,
                    bias=bias_t[:, :], scale=1.0,
                )
                nc.sync.dma_start(out=out[g, :, 0], in_=s_a[:, :, :, :])
                nc.scalar.activation(
                    s_b[:, :, :, :], q2_b[:, :, :, :],
                    mybir.ActivationFunctionType.Relu,
                    bias=bias_t[:, :], scale=1.0,
                )
                nc.sync.dma_start(out=out[g, :, 1], in_=s_b[:, :, :, :])
                q_a, q_b = q2_a, q2_b
    nc.compile()
    return nc

# --------------------------------------------------------------- host side

def _prep_weights(conv_w, fc_w, fc_b):
    """W_aug permuted to [128 p, KT, NJ, 128 m]: row kt*128+p = pixel (or bias
    at 784, zero pad above), col j*128+m = feature; scaled by 0.5 (tau fold)."""
    cw = conv_w.reshape(2, 2).astype(np.float32)
    fcw = fc_w.astype(np.float32).reshape(NF, 27, 27)
    tmp = np.zeros((NF, H, W), np.float32)
    for dr in range(2):
        for dc in range(2):
            tmp[:, dr:dr + 27, dc:dc + 27] += cw[dr, dc] * fcw
    w_eff = tmp.reshape(NF, NPIX)                     # [729, 784]
    w_aug = np.zeros((KT * 128, NJ * 128), np.float32)
    w_aug[:NPIX, :NF] = 0.5 * w_eff.T
    w_aug[NPIX, :NF] = 0.5 * fc_b.astype(np.float32)
    # [KT*128, NJ*128] -> [KT, 128, NJ, 128] -> [128, NJ, KT, 128]
    return np.ascontiguousarray(
        w_aug.reshape(KT, 128, NJ, 128).transpose(1, 2, 0, 3))

def _prep_x(x_seq):
    """Per-core pixel-major inputs [NCORES][NG, 128, KT, NS]."""
    xs = np.ascontiguousarray(x_seq.reshape(T, NCORES, BS, NPIX))
    # -> [core, group, pixel, (tl, sample)]
    xt = xs.transpose(1, 0, 3, 2).reshape(NCORES, NG, G, NPIX, BS)
    xt = xt.transpose(0, 1, 3, 2, 4).reshape(NCORES, NG, NPIX, NS)
    xp = np.zeros((NCORES, NG, KT * 128, NS), np.float32)
    xp[:, :, :NPIX, :] = xt
    xp[:, :, NPIX, :] = 1.0
    # rows -> [128, KT]: row kt*128+p at [p, kt]
    xp = xp.reshape(NCORES, NG, KT, 128, NS).transpose(0, 1, 3, 2, 4)
    return np.ascontiguousarray(xp)

def kernel(x_seq, conv_w, fc_w, fc_b):
    if "nc" not in _CACHE:
        _CACHE["nc"] = _build()
    nc = _CACHE["nc"]
    w_aug = _prep_weights(conv_w, fc_w, fc_b)
    xp = _prep_x(np.asarray(x_seq, dtype=np.float32))
    in_maps = [{"x_in": np.ascontiguousarray(xp[c]), "w_in": w_aug}
               for c in range(NCORES)]
    res = run_bass_kernel_spmd(nc, in_maps, core_ids=list(range(NCORES)))
    _CACHE["last_result"] = res
    full = np.empty((T, B, NF), np.float32)
    for c in range(NCORES):
        o = res.results[c]["out"]             # [NG, 128, 2, G, NJH, BS] u8
        # (g, p, h, tl, jh, s) -> (g, tl, s, h, jh, p); f = (h*NJH+jh)*128+p
        full[:, c * BS:(c + 1) * BS, :] = (
            o.transpose(0, 3, 5, 2, 4, 1).reshape(T, BS, NJ * 128)[:, :, :NF]
            .astype(np.float32))
    return full
